# revision 27
# baseline (speedup 1.0000x reference)
"""Trainium2 Bass kernel for the bidirectional Mamba-style selective scan.

Problem (B=4, L=2048, D=256, N=16, dt_rank=16):
    x_dbl   = x @ Wx.T                      -> delta_r | Bf | Bb | C
    db_r    = flip(x) @ Wxb.T
    delta   = softplus(delta_r @ Wdt.T + bdt)
    delta_b = softplus(db_r    @ Wdt.T + bdt)
    dA      = exp(delta outer A)            (A = -exp(A_log))
    dBu     = delta*Bf*x + delta_b*Bb*flip(x)
    h_t     = dA_t * h_{t-1} + dBu_t        (scan over L)
    y_t     = einsum(h_t, C_t) ; y += (x + flip(x)) * Dp
    out     = (y @ Wout.T) @ Wadapt.T       -> (B, L)
    returns (out, x)

Sharding: 8 cores = 4 batches x 2 halves of d_inner (128 each).  out_proj is
immediately contracted by Wadapt to one channel, so we fold v = Wadapt @ Wout
(one 256-vector); each core produces y_part[t] = sum_{d in half} v[d]*y[t,d]
and the host adds the two halves per batch (plus the skip term, computed on
host: it is a single dot product with the input).

Per-core device program (d on partitions, t on the free axis, fp32 data with
float32r matmuls -- full-rate on the PE):
  - PE: projections; per-n partition-broadcast of Bf/Bb rows (one-hot selector
    matmul); per-n contraction G_n[t] = sum_d v[d] H_n[d,t] to a (1, 512)
    PSUM row.
  - ACT: softplus (exp + ln -- one shared table set), per-n exp with
    per-partition scale A[:,n]; gathers G rows into partition 32*(n%4) of an
    SBUF tile (engine partition shifts are legal at 32-aligned bases).
  - DVE: dBu products, tensor_tensor_scan recurrence, the small C*G stage.

The sequence flip never happens on device: the host passes both x^T and
flip(x)^T, and every "backward" quantity is computed at forward time from the
flipped copy.
"""

import numpy as np

try:
    import concourse.bass as bass
except ImportError:  # fresh grading dir: repo not on sys.path
    import sys

    sys.path.insert(0, "/opt/trn_rl_repo")
    import concourse.bass as bass

from contextlib import ExitStack

import concourse.bacc as bacc
import concourse.mybir as mybir
import concourse.tile as tile
from concourse.bass_utils import run_bass_kernel_spmd
from concourse.tile_rust import add_dep_helper

L = 2048  # sequence length
DH = 128  # d_inner half handled per core
NST = 16  # state dim n
RK = 16  # dt_rank
CH = 512  # chunk size (one fp32 PSUM bank)
NCH = L // CH

F32 = mybir.dt.float32
F32R = mybir.dt.float32r
AF = mybir.ActivationFunctionType
ALU = mybir.AluOpType


def _f(ap):
    """Reinterpret a float32r AP back as plain fp32 for DVE/ACT reads."""
    return ap.bitcast(mybir.dt.float32)


_BUILT = None


def _build():
    global _BUILT
    if _BUILT is not None:
        return _BUILT

    nc = bacc.Bacc(None, target_bir_lowering=False)

    din = {}
    # wa/wb cols: [0:48 Wx.T | 48:64 Wxb.T | 64:576 wxc] for the two d-halves;
    # w16 cols: [0:2048 one-hot selectors | 2048:2176 Wdt.T];
    # cvec cols: [0 bdt | 1:17 A | 17 v | 18 ones].
    for name, shape in [
        ("xt_a", [DH, L]),  # own d-half of x[b].T
        ("xt_b", [DH, L]),  # other d-half
        ("xr_a", [DH, L]),  # own d-half of flip(x[b]).T
        ("xr_b", [DH, L]),
        ("wa", [DH, 576]),
        ("wb", [DH, 576]),
        ("w16", [NST, NST * DH + DH]),
        ("cvec", [DH, 19]),
    ]:
        din[name] = nc.dram_tensor(name, shape, F32R, kind="ExternalInput")
    y_part = nc.dram_tensor("y_part", [1, L], F32, kind="ExternalOutput")

    with tile.TileContext(nc) as tc, ExitStack() as ctx:
        const = ctx.enter_context(tc.tile_pool(name="const", bufs=1))
        big = ctx.enter_context(tc.tile_pool(name="big", bufs=1))
        work = ctx.enter_context(tc.tile_pool(name="work", bufs=2))

        wts = {}
        for name in ["wa", "wb", "w16", "cvec"]:
            dram = din[name]
            t = const.tile(list(dram.shape), dram.dtype, tag=name, name=f"sb_{name}")
            nc.sync.dma_start(t[:], dram[:])
            wts[name] = t
        wxa, wxb_a, wxca = wts["wa"][:, 0:48], wts["wa"][:, 48:64], wts["wa"][:, 64:576]
        wxb, wxb_b, wxcb = wts["wb"][:, 0:48], wts["wb"][:, 48:64], wts["wb"][:, 64:576]
        sel_all, wdt = wts["w16"][:, 0 : NST * DH], wts["w16"][:, NST * DH :]
        bdt_ap = _f(wts["cvec"][:, 0:1])
        amat = wts["cvec"][:, 1 : 1 + NST]
        vvec = wts["cvec"][:, 17:18]
        ones1 = wts["cvec"][:, 18:19]

        stmp = tc.alloc_tile_pool(name="stmp", bufs=1)
        xs = {}
        for name in ["xt_a", "xt_b", "xr_a", "xr_b"]:
            pool = big if name in ("xt_a", "xr_a") else stmp
            t = pool.tile([DH, L], F32R, tag=name, name=f"sb_{name}")
            nc.sync.dma_start(t[:], din[name][:])
            xs[name] = t

        bfT = big.tile([NST, L], F32R, tag="bfT")
        bbT = big.tile([NST, L], F32R, tag="bbT")
        deltaT = big.tile([DH, L], F32, tag="deltaT")
        pf = big.tile([DH, L], F32, tag="pf")
        pb = big.tile([DH, L], F32, tag="pb")
        # C rows: group g at partitions {0,32,64,96}; built directly by the
        # wxc projection so the unused partitions are exact zeros.
        ctg = [big.tile([DH, L], F32, tag=f"ctg{g}", name=f"ctg{g}") for g in range(4)]

        with tc.tile_pool(name="psetup", bufs=2, space=bass.MemorySpace.PSUM) as psetup:
            spl = stmp.tile([DH, L], F32, tag="spl", name="spl", bufs=1)
            for c in range(NCH):
                cs = slice(c * CH, (c + 1) * CH)
                for gi, dst in enumerate([None, bfT, bbT]):
                    gs = slice(gi * 16, (gi + 1) * 16)
                    p_xdbl = psetup.tile([16, CH], F32, tag="p_xdbl")
                    nc.tensor.matmul(p_xdbl[:], wxa[:, gs], xs["xt_a"][:, cs], start=True, stop=False)
                    nc.tensor.matmul(p_xdbl[:], wxb[:, gs], xs["xt_b"][:, cs], start=False, stop=True)
                    if dst is None:
                        drT_c = stmp.tile([RK, CH], F32R, tag="drT", name="drT_c", bufs=2)
                        nc.scalar.copy(drT_c[:], p_xdbl[:])
                        p_dd = psetup.tile([DH, CH], F32, tag="p_dd")
                        nc.tensor.matmul(p_dd[:], wdt[:], drT_c[:], start=True, stop=True)
                        nc.scalar.activation(spl[:, cs], p_dd[:], AF.Exp, bias=bdt_ap, scale=1.0)
                    else:
                        nc.scalar.copy(dst[:, cs], p_xdbl[:])
            for c in range(NCH):
                cs = slice(c * CH, (c + 1) * CH)
                nc.scalar.activation(deltaT[:, cs], spl[:, cs], AF.Ln, bias=1.0, scale=1.0)
            for g in range(4):
                ggs = slice(g * DH, (g + 1) * DH)
                for c in range(NCH):
                    cs = slice(c * CH, (c + 1) * CH)
                    p_ct = psetup.tile([DH, CH], F32, tag="p_ct")
                    nc.tensor.matmul(p_ct[:], wxca[:, ggs], xs["xt_a"][:, cs], start=True, stop=False)
                    nc.tensor.matmul(p_ct[:], wxcb[:, ggs], xs["xt_b"][:, cs], start=False, stop=True)
                    nc.scalar.copy(ctg[g][:, cs], p_ct[:])
            # softplus(z) = ln(exp(z) + 1); exp and ln live in one ACT table
            # set (no softplus table exists in this compiler).
            for c in range(NCH):
                cs = slice(c * CH, (c + 1) * CH)
                p_dbr = psetup.tile([RK, CH], F32, tag="p_dbr")
                nc.tensor.matmul(p_dbr[:], wxb_a[:], xs["xr_a"][:, cs], start=True, stop=False)
                nc.tensor.matmul(p_dbr[:], wxb_b[:], xs["xr_b"][:, cs], start=False, stop=True)
                dbr_c = stmp.tile([RK, CH], F32R, tag="dbr", name="dbr_c", bufs=2)
                nc.scalar.copy(dbr_c[:], p_dbr[:])
                p_dd = psetup.tile([DH, CH], F32, tag="p_dd")
                nc.tensor.matmul(p_dd[:], wdt[:], dbr_c[:], start=True, stop=True)
                nc.scalar.activation(spl[:, cs], p_dd[:], AF.Exp, bias=bdt_ap, scale=1.0)
            last_ln = None
            for c in range(NCH):
                cs = slice(c * CH, (c + 1) * CH)
                dlb_c = stmp.tile([DH, CH], F32, tag="dlb", name="dlb_c", bufs=2)
                last_ln = nc.scalar.activation(dlb_c[:], spl[:, cs], AF.Ln, bias=1.0, scale=1.0)
                nc.vector.tensor_mul(pb[:, cs], dlb_c[:], _f(xs["xr_a"][:, cs]))

        nc.vector.tensor_mul(pf[:], deltaT[:], _f(xs["xt_a"][:]))

        ysb = big.tile([1, L], F32, tag="ysb")
        nc.vector.memset(ysb[:], 0.0)
        stmp.release()

        with (
            tc.tile_pool(name="pbc", bufs=2, space=bass.MemorySpace.PSUM) as pbc,
            tc.tile_pool(name="pgp", bufs=2, space=bass.MemorySpace.PSUM) as pgp,
            tc.tile_pool(name="pyg", bufs=2, space=bass.MemorySpace.PSUM) as pyg,
        ):
            # G rows gather tile: group g writes rows {0,32,64,96}; all other
            # rows are zeroed once and multiply against exact-zero ctg rows.
            gsb = work.tile([DH, L], F32, tag="gsb", name="gsb", bufs=1)
            nc.vector.memset(gsb[:], 0.0)
            cg = work.tile([DH, L], F32R, tag="cg", name="cg", bufs=1)
            for g in range(4):
                for k in range(4):
                    n = 4 * g + k
                    seln = sel_all[:, n * DH : (n + 1) * DH]
                    h_prev = None
                    for j in range(2):
                        FD = 2 * CH
                        js = slice(j * FD, (j + 1) * FD)
                        dA = work.tile([DH, FD], F32, tag="dA", name="dA", bufs=4)
                        exp_i = nc.scalar.activation(
                            dA[:], deltaT[:, js], AF.Exp, bias=0.0, scale=_f(amat[:, n : n + 1])
                        )
                        if n == 0 and j == 0 and last_ln is not None:
                            add_dep_helper(exp_i.ins, last_ln.ins, reason="act-table batching")
                        bcf = pbc.tile([DH, FD], F32, tag="bc", name="bcf")
                        for h in range(2):
                            hs = slice(h * CH, (h + 1) * CH)
                            nc.tensor.matmul(bcf[:, hs], seln, bfT[:, j * FD + h * CH : j * FD + (h + 1) * CH])
                        w1 = work.tile([DH, FD], F32, tag="w1", name="w1")
                        nc.vector.tensor_mul(w1[:], pf[:, js], bcf[:])
                        bcb = pbc.tile([DH, FD], F32, tag="bc", name="bcb")
                        for h in range(2):
                            hs = slice(h * CH, (h + 1) * CH)
                            nc.tensor.matmul(bcb[:, hs], seln, bbT[:, j * FD + h * CH : j * FD + (h + 1) * CH])
                        w2 = work.tile([DH, FD], F32, tag="w2", name="w2")
                        nc.vector.tensor_mul(w2[:], pb[:, js], bcb[:])
                        dbu = work.tile([DH, FD], F32, tag="dbu", name="dbu")
                        nc.vector.tensor_add(dbu[:], w1[:], w2[:])
                        for h in range(2):
                            c = 2 * j + h
                            cs = slice(c * CH, (c + 1) * CH)
                            hhs = slice(h * CH, (h + 1) * CH)
                            hs_t = work.tile([DH, CH], F32R, tag="h", name="hst", bufs=3)
                            init = 0.0 if c == 0 else _f(h_prev[:, CH - 1 : CH])
                            nc.vector.tensor_tensor_scan(
                                hs_t[:], dA[:, hhs], dbu[:, hhs], init, ALU.mult, ALU.add
                            )
                            h_prev = hs_t
                            gp1 = pgp.tile([1, CH], F32, tag="gp1", name="gp1")
                            nc.tensor.matmul(gp1[:], vvec, hs_t[:])
                            nc.scalar.copy(gsb[32 * k : 32 * k + 1, cs], gp1[:])
                # multiply the gathered G rows by C, reduce over d and n
                nc.vector.tensor_mul(cg[:], ctg[g][:], gsb[:])
                for c in range(NCH):
                    cs = slice(c * CH, (c + 1) * CH)
                    yg = pyg.tile([1, CH], F32, tag="yg", name=f"yg{g}_{c}")
                    nc.tensor.matmul(yg[:], ones1, cg[:, cs])
                    nc.vector.tensor_add(ysb[:, cs], ysb[:, cs], yg[:])

            nc.sync.dma_start(y_part[:], ysb[:])

    nc.compile()
    _BUILT = nc
    return nc


def _shard_inputs(x, Wx, Wxb, Wdt, bdt, A_log, Dp, Wout, Wadapt):
    x = np.asarray(x, dtype=np.float32)
    Wx = np.asarray(Wx, dtype=np.float32)
    Wxb = np.asarray(Wxb, dtype=np.float32)
    Wdt = np.asarray(Wdt, dtype=np.float32)
    bdt = np.asarray(bdt, dtype=np.float32)
    A = -np.exp(np.asarray(A_log, dtype=np.float32))  # (D, N)
    Wout = np.asarray(Wout, dtype=np.float32)
    Wadapt = np.asarray(Wadapt, dtype=np.float32)

    v = (Wadapt @ Wout)[0]  # (D,)
    WxT = np.ascontiguousarray(Wx.T)  # (D, 64)
    WxbT = np.ascontiguousarray(Wxb.T)  # (D, R)
    WdtT = np.ascontiguousarray(Wdt.T)  # (R, D)
    # wxc[d, g*128 + 32*k] = Wx[48 + 4g + k, d]
    wxc = np.zeros((2 * DH, 4 * DH), dtype=np.float32)
    for g in range(4):
        for k in range(4):
            wxc[:, g * DH + 32 * k] = Wx[48 + 4 * g + k, :]

    in_maps = []
    for cidx in range(8):
        bi, hi = cidx // 2, cidx % 2
        sa = slice(hi * DH, hi * DH + DH)
        sb = slice((1 - hi) * DH, (1 - hi) * DH + DH)
        xT = np.ascontiguousarray(x[bi].T)  # (D, L)
        xrT = np.ascontiguousarray(x[bi, ::-1].T)
        wa = np.concatenate([WxT[sa, :48], WxbT[sa], wxc[sa]], axis=1)
        wb = np.concatenate([WxT[sb, :48], WxbT[sb], wxc[sb]], axis=1)
        w16 = np.zeros((NST, NST * DH + DH), dtype=np.float32)
        for n in range(NST):
            w16[n, n * DH : (n + 1) * DH] = 1.0
        w16[:, NST * DH :] = WdtT[:, sa]
        cvec = np.zeros((DH, 19), dtype=np.float32)
        cvec[:, 0] = bdt[sa]
        cvec[:, 1 : 1 + NST] = A[sa]
        cvec[:, 17] = v[sa]
        cvec[:, 18] = 1.0
        in_maps.append(
            {
                "xt_a": np.ascontiguousarray(xT[sa]),
                "xt_b": np.ascontiguousarray(xT[sb]),
                "xr_a": np.ascontiguousarray(xrT[sa]),
                "xr_b": np.ascontiguousarray(xrT[sb]),
                "wa": np.ascontiguousarray(wa),
                "wb": np.ascontiguousarray(wb),
                "w16": w16,
                "cvec": cvec,
            }
        )
    return in_maps


def kernel(x, Wx, Wxb, Wdt, bdt, A_log, Dp, Wout, Wadapt, _run_kwargs=None):
    nc = _build()
    x = np.asarray(x, dtype=np.float32)
    in_maps = _shard_inputs(x, Wx, Wxb, Wdt, bdt, A_log, Dp, Wout, Wadapt)
    kwargs = dict(_run_kwargs or {})
    res = run_bass_kernel_spmd(nc, in_maps, core_ids=list(range(8)), **kwargs)
    parts = [r["y_part"].reshape(L) for r in res.results]
    # skip path (y += (x + flip(x)) * Dp before out_proj) reduces to one dot
    # with vd = (Wadapt @ Wout) * Dp; computed on host.
    v = (np.asarray(Wadapt, np.float32) @ np.asarray(Wout, np.float32))[0]
    vd = v * np.asarray(Dp, np.float32)
    y = np.empty((4, L), dtype=np.float32)
    for b in range(4):
        t = x[b] @ vd
        y[b] = parts[2 * b] + parts[2 * b + 1] + t + t[::-1]
    out = (y, x)
    if _run_kwargs is not None:
        return out, res
    return out


# revision 29
# speedup vs baseline: 1.0088x; 1.0088x over previous
"""Trainium2 Bass kernel for the bidirectional Mamba-style selective scan.

Problem (B=4, L=2048, D=256, N=16, dt_rank=16):
    x_dbl   = x @ Wx.T                      -> delta_r | Bf | Bb | C
    db_r    = flip(x) @ Wxb.T
    delta   = softplus(delta_r @ Wdt.T + bdt)
    delta_b = softplus(db_r    @ Wdt.T + bdt)
    dA      = exp(delta outer A)            (A = -exp(A_log))
    dBu     = delta*Bf*x + delta_b*Bb*flip(x)
    h_t     = dA_t * h_{t-1} + dBu_t        (scan over L)
    y_t     = einsum(h_t, C_t) ; y += (x + flip(x)) * Dp
    out     = (y @ Wout.T) @ Wadapt.T       -> (B, L)
    returns (out, x)

Sharding: 8 cores = 4 batches x 2 halves of d_inner (128 each).  out_proj is
immediately contracted by Wadapt to one channel, so we fold v = Wadapt @ Wout
(one 256-vector); each core produces y_part[t] = sum_{d in half} v[d]*y[t,d]
and the host adds the two halves per batch (plus the skip term, computed on
host: it is a single dot product with the input).

Per-core device program (d on partitions, t on the free axis, fp32 data with
float32r matmuls -- full-rate on the PE):
  - PE: projections; per-n partition-broadcast of Bf/Bb rows (one-hot selector
    matmul); per-n contraction G_n[t] = sum_d v[d] H_n[d,t] to a (1, 512)
    PSUM row.
  - ACT: softplus (exp + ln -- one shared table set), per-n exp with
    per-partition scale A[:,n]; gathers G rows into partition 32*(n%4) of an
    SBUF tile (engine partition shifts are legal at 32-aligned bases).
  - DVE: dBu products, tensor_tensor_scan recurrence, the small C*G stage.

The sequence flip never happens on device: the host passes both x^T and
flip(x)^T, and every "backward" quantity is computed at forward time from the
flipped copy.
"""

import numpy as np

try:
    import concourse.bass as bass
except ImportError:  # fresh grading dir: repo not on sys.path
    import sys

    sys.path.insert(0, "/opt/trn_rl_repo")
    import concourse.bass as bass

from contextlib import ExitStack

import concourse.bacc as bacc
import concourse.mybir as mybir
import concourse.tile as tile
from concourse.bass_utils import run_bass_kernel_spmd
from concourse.tile_rust import add_dep_helper

L = 2048  # sequence length
DH = 128  # d_inner half handled per core
NST = 16  # state dim n
RK = 16  # dt_rank
CH = 512  # chunk size (one fp32 PSUM bank)
NCH = L // CH

F32 = mybir.dt.float32
F32R = mybir.dt.float32r
AF = mybir.ActivationFunctionType
ALU = mybir.AluOpType


def _f(ap):
    """Reinterpret a float32r AP back as plain fp32 for DVE/ACT reads."""
    return ap.bitcast(mybir.dt.float32)


_BUILT = None


def _build():
    global _BUILT
    if _BUILT is not None:
        return _BUILT

    nc = bacc.Bacc(None, target_bir_lowering=False)

    din = {}
    # wa/wb cols: [0:48 Wx.T | 48:64 Wxb.T | 64:576 wxc] for the two d-halves;
    # w16 cols: [0:2048 one-hot selectors | 2048:2176 Wdt.T];
    # cvec cols: [0 bdt | 1:17 A | 17 v | 18 ones].
    for name, shape in [
        ("xt_a", [DH, L]),  # own d-half of x[b].T
        ("xt_b", [DH, L]),  # other d-half
        ("xr_a", [DH, L]),  # own d-half of flip(x[b]).T
        ("xr_b", [DH, L]),
        ("wa", [DH, 576]),
        ("wb", [DH, 576]),
        ("w16", [NST, NST * DH + DH]),
        ("cvec", [DH, 19]),
    ]:
        din[name] = nc.dram_tensor(name, shape, F32R, kind="ExternalInput")
    y_part = nc.dram_tensor("y_part", [1, L], F32, kind="ExternalOutput")

    with tile.TileContext(nc) as tc, ExitStack() as ctx:
        const = ctx.enter_context(tc.tile_pool(name="const", bufs=1))
        big = ctx.enter_context(tc.tile_pool(name="big", bufs=1))
        work = ctx.enter_context(tc.tile_pool(name="work", bufs=2))

        wts = {}
        for name in ["wa", "wb", "w16", "cvec"]:
            dram = din[name]
            t = const.tile(list(dram.shape), dram.dtype, tag=name, name=f"sb_{name}")
            nc.sync.dma_start(t[:], dram[:])
            wts[name] = t
        wxa, wxb_a, wxca = wts["wa"][:, 0:48], wts["wa"][:, 48:64], wts["wa"][:, 64:576]
        wxb, wxb_b, wxcb = wts["wb"][:, 0:48], wts["wb"][:, 48:64], wts["wb"][:, 64:576]
        sel_all, wdt = wts["w16"][:, 0 : NST * DH], wts["w16"][:, NST * DH :]
        bdt_ap = _f(wts["cvec"][:, 0:1])
        amat = wts["cvec"][:, 1 : 1 + NST]
        vvec = wts["cvec"][:, 17:18]
        ones1 = wts["cvec"][:, 18:19]

        stmp = tc.alloc_tile_pool(name="stmp", bufs=1)
        xs = {}
        for name in ["xt_a", "xt_b", "xr_a", "xr_b"]:
            pool = big if name in ("xt_a", "xr_a") else stmp
            t = pool.tile([DH, L], F32R, tag=name, name=f"sb_{name}")
            nc.sync.dma_start(t[:], din[name][:])
            xs[name] = t

        bfT = big.tile([NST, L], F32R, tag="bfT")
        bbT = big.tile([NST, L], F32R, tag="bbT")
        deltaT = big.tile([DH, L], F32, tag="deltaT")
        pf = big.tile([DH, L], F32, tag="pf")
        pb = big.tile([DH, L], F32, tag="pb")
        # C rows: group g at partitions {0,32,64,96}; built directly by the
        # wxc projection so the unused partitions are exact zeros.
        ctg = [big.tile([DH, L], F32, tag=f"ctg{g}", name=f"ctg{g}") for g in range(4)]

        with tc.tile_pool(name="psetup", bufs=2, space=bass.MemorySpace.PSUM) as psetup:
            spl = stmp.tile([DH, L], F32, tag="spl", name="spl", bufs=1)
            for c in range(NCH):
                cs = slice(c * CH, (c + 1) * CH)
                for gi, dst in enumerate([None, bfT, bbT]):
                    gs = slice(gi * 16, (gi + 1) * 16)
                    p_xdbl = psetup.tile([16, CH], F32, tag="p_xdbl")
                    nc.tensor.matmul(p_xdbl[:], wxa[:, gs], xs["xt_a"][:, cs], start=True, stop=False)
                    nc.tensor.matmul(p_xdbl[:], wxb[:, gs], xs["xt_b"][:, cs], start=False, stop=True)
                    if dst is None:
                        drT_c = stmp.tile([RK, CH], F32R, tag="drT", name="drT_c", bufs=2)
                        nc.scalar.copy(drT_c[:], p_xdbl[:])
                        p_dd = psetup.tile([DH, CH], F32, tag="p_dd")
                        nc.tensor.matmul(p_dd[:], wdt[:], drT_c[:], start=True, stop=True)
                        last_fexp = nc.scalar.activation(spl[:, cs], p_dd[:], AF.Exp, bias=bdt_ap, scale=1.0)
                    else:
                        nc.scalar.copy(dst[:, cs], p_xdbl[:])
            for c in range(NCH):
                cs = slice(c * CH, (c + 1) * CH)
                ln_i = nc.scalar.activation(deltaT[:, cs], spl[:, cs], AF.Ln, bias=1.0, scale=1.0)
                if c == 0:
                    add_dep_helper(ln_i.ins, last_fexp.ins, reason="act-table batching fwd")
            for g in range(4):
                ggs = slice(g * DH, (g + 1) * DH)
                for c in range(NCH):
                    cs = slice(c * CH, (c + 1) * CH)
                    p_ct = psetup.tile([DH, CH], F32, tag="p_ct")
                    nc.tensor.matmul(p_ct[:], wxca[:, ggs], xs["xt_a"][:, cs], start=True, stop=False)
                    nc.tensor.matmul(p_ct[:], wxcb[:, ggs], xs["xt_b"][:, cs], start=False, stop=True)
                    nc.scalar.copy(ctg[g][:, cs], p_ct[:])
            # softplus(z) = ln(exp(z) + 1); exp and ln live in one ACT table
            # set (no softplus table exists in this compiler).
            for c in range(NCH):
                cs = slice(c * CH, (c + 1) * CH)
                p_dbr = psetup.tile([RK, CH], F32, tag="p_dbr")
                nc.tensor.matmul(p_dbr[:], wxb_a[:], xs["xr_a"][:, cs], start=True, stop=False)
                nc.tensor.matmul(p_dbr[:], wxb_b[:], xs["xr_b"][:, cs], start=False, stop=True)
                dbr_c = stmp.tile([RK, CH], F32R, tag="dbr", name="dbr_c", bufs=2)
                nc.scalar.copy(dbr_c[:], p_dbr[:])
                p_dd = psetup.tile([DH, CH], F32, tag="p_dd")
                nc.tensor.matmul(p_dd[:], wdt[:], dbr_c[:], start=True, stop=True)
                last_bexp = nc.scalar.activation(spl[:, cs], p_dd[:], AF.Exp, bias=bdt_ap, scale=1.0)
            last_ln = None
            for c in range(NCH):
                cs = slice(c * CH, (c + 1) * CH)
                dlb_c = stmp.tile([DH, CH], F32, tag="dlb", name="dlb_c", bufs=2)
                last_ln = nc.scalar.activation(dlb_c[:], spl[:, cs], AF.Ln, bias=1.0, scale=1.0)
                if c == 0:
                    add_dep_helper(last_ln.ins, last_bexp.ins, reason="act-table batching bwd")
                nc.vector.tensor_mul(pb[:, cs], dlb_c[:], _f(xs["xr_a"][:, cs]))

        nc.vector.tensor_mul(pf[:], deltaT[:], _f(xs["xt_a"][:]))

        ysb = big.tile([1, L], F32, tag="ysb")
        nc.vector.memset(ysb[:], 0.0)
        stmp.release()

        with (
            tc.tile_pool(name="pbc", bufs=2, space=bass.MemorySpace.PSUM) as pbc,
            tc.tile_pool(name="pgp", bufs=2, space=bass.MemorySpace.PSUM) as pgp,
            tc.tile_pool(name="pyg", bufs=2, space=bass.MemorySpace.PSUM) as pyg,
        ):
            # G rows gather tile: group g writes rows {0,32,64,96}; all other
            # rows are zeroed once and multiply against exact-zero ctg rows.
            gsb = work.tile([DH, L], F32, tag="gsb", name="gsb", bufs=1)
            nc.vector.memset(gsb[:], 0.0)
            cg = work.tile([DH, L], F32R, tag="cg", name="cg", bufs=1)
            for g in range(4):
                for k in range(4):
                    n = 4 * g + k
                    seln = sel_all[:, n * DH : (n + 1) * DH]
                    h_prev = None
                    for j in range(2):
                        FD = 2 * CH
                        js = slice(j * FD, (j + 1) * FD)
                        dA = work.tile([DH, FD], F32, tag="dA", name="dA")
                        exp_i = nc.scalar.activation(
                            dA[:], deltaT[:, js], AF.Exp, bias=0.0, scale=_f(amat[:, n : n + 1])
                        )
                        if n == 0 and j == 0 and last_ln is not None:
                            add_dep_helper(exp_i.ins, last_ln.ins, reason="act-table batching")
                        bcf = pbc.tile([DH, FD], F32, tag="bc", name="bcf")
                        for h in range(2):
                            hs = slice(h * CH, (h + 1) * CH)
                            nc.tensor.matmul(bcf[:, hs], seln, bfT[:, j * FD + h * CH : j * FD + (h + 1) * CH])
                        w1 = work.tile([DH, FD], F32, tag="w1", name="w1")
                        nc.vector.tensor_mul(w1[:], pf[:, js], bcf[:])
                        bcb = pbc.tile([DH, FD], F32, tag="bc", name="bcb")
                        for h in range(2):
                            hs = slice(h * CH, (h + 1) * CH)
                            nc.tensor.matmul(bcb[:, hs], seln, bbT[:, j * FD + h * CH : j * FD + (h + 1) * CH])
                        w2 = work.tile([DH, FD], F32, tag="w2", name="w2")
                        nc.vector.tensor_mul(w2[:], pb[:, js], bcb[:])
                        dbu = work.tile([DH, FD], F32, tag="dbu", name="dbu")
                        nc.vector.tensor_add(dbu[:], w1[:], w2[:])
                        hs_t = work.tile([DH, FD], F32R, tag="h", name="hst")
                        init = 0.0 if j == 0 else _f(h_prev[:, FD - 1 : FD])
                        nc.vector.tensor_tensor_scan(
                            hs_t[:], dA[:], dbu[:], init, ALU.mult, ALU.add
                        )
                        h_prev = hs_t
                        for h in range(2):
                            c = 2 * j + h
                            cs = slice(c * CH, (c + 1) * CH)
                            hhs = slice(h * CH, (h + 1) * CH)
                            gp1 = pgp.tile([1, CH], F32, tag="gp1", name="gp1")
                            nc.tensor.matmul(gp1[:], vvec, hs_t[:, hhs])
                            nc.scalar.copy(gsb[32 * k : 32 * k + 1, cs], gp1[:])
                # multiply the gathered G rows by C, reduce over d and n
                nc.vector.tensor_mul(cg[:], ctg[g][:], gsb[:])
                for c in range(NCH):
                    cs = slice(c * CH, (c + 1) * CH)
                    yg = pyg.tile([1, CH], F32, tag="yg", name=f"yg{g}_{c}")
                    nc.tensor.matmul(yg[:], ones1, cg[:, cs])
                    nc.vector.tensor_add(ysb[:, cs], ysb[:, cs], yg[:])

            nc.sync.dma_start(y_part[:], ysb[:])

    nc.compile()
    _BUILT = nc
    return nc


def _shard_inputs(x, Wx, Wxb, Wdt, bdt, A_log, Dp, Wout, Wadapt):
    x = np.asarray(x, dtype=np.float32)
    Wx = np.asarray(Wx, dtype=np.float32)
    Wxb = np.asarray(Wxb, dtype=np.float32)
    Wdt = np.asarray(Wdt, dtype=np.float32)
    bdt = np.asarray(bdt, dtype=np.float32)
    A = -np.exp(np.asarray(A_log, dtype=np.float32))  # (D, N)
    Wout = np.asarray(Wout, dtype=np.float32)
    Wadapt = np.asarray(Wadapt, dtype=np.float32)

    v = (Wadapt @ Wout)[0]  # (D,)
    WxT = np.ascontiguousarray(Wx.T)  # (D, 64)
    WxbT = np.ascontiguousarray(Wxb.T)  # (D, R)
    WdtT = np.ascontiguousarray(Wdt.T)  # (R, D)
    # wxc[d, g*128 + 32*k] = Wx[48 + 4g + k, d]
    wxc = np.zeros((2 * DH, 4 * DH), dtype=np.float32)
    for g in range(4):
        for k in range(4):
            wxc[:, g * DH + 32 * k] = Wx[48 + 4 * g + k, :]

    in_maps = []
    for cidx in range(8):
        bi, hi = cidx // 2, cidx % 2
        sa = slice(hi * DH, hi * DH + DH)
        sb = slice((1 - hi) * DH, (1 - hi) * DH + DH)
        xT = np.ascontiguousarray(x[bi].T)  # (D, L)
        xrT = np.ascontiguousarray(x[bi, ::-1].T)
        wa = np.concatenate([WxT[sa, :48], WxbT[sa], wxc[sa]], axis=1)
        wb = np.concatenate([WxT[sb, :48], WxbT[sb], wxc[sb]], axis=1)
        w16 = np.zeros((NST, NST * DH + DH), dtype=np.float32)
        for n in range(NST):
            w16[n, n * DH : (n + 1) * DH] = 1.0
        w16[:, NST * DH :] = WdtT[:, sa]
        cvec = np.zeros((DH, 19), dtype=np.float32)
        cvec[:, 0] = bdt[sa]
        cvec[:, 1 : 1 + NST] = A[sa]
        cvec[:, 17] = v[sa]
        cvec[:, 18] = 1.0
        in_maps.append(
            {
                "xt_a": np.ascontiguousarray(xT[sa]),
                "xt_b": np.ascontiguousarray(xT[sb]),
                "xr_a": np.ascontiguousarray(xrT[sa]),
                "xr_b": np.ascontiguousarray(xrT[sb]),
                "wa": np.ascontiguousarray(wa),
                "wb": np.ascontiguousarray(wb),
                "w16": w16,
                "cvec": cvec,
            }
        )
    return in_maps


def kernel(x, Wx, Wxb, Wdt, bdt, A_log, Dp, Wout, Wadapt, _run_kwargs=None):
    nc = _build()
    x = np.asarray(x, dtype=np.float32)
    in_maps = _shard_inputs(x, Wx, Wxb, Wdt, bdt, A_log, Dp, Wout, Wadapt)
    kwargs = dict(_run_kwargs or {})
    res = run_bass_kernel_spmd(nc, in_maps, core_ids=list(range(8)), **kwargs)
    parts = [r["y_part"].reshape(L) for r in res.results]
    # skip path (y += (x + flip(x)) * Dp before out_proj) reduces to one dot
    # with vd = (Wadapt @ Wout) * Dp; computed on host.
    v = (np.asarray(Wadapt, np.float32) @ np.asarray(Wout, np.float32))[0]
    vd = v * np.asarray(Dp, np.float32)
    y = np.empty((4, L), dtype=np.float32)
    for b in range(4):
        t = x[b] @ vd
        y[b] = parts[2 * b] + parts[2 * b + 1] + t + t[::-1]
    out = (y, x)
    if _run_kwargs is not None:
        return out, res
    return out


# revision 30
# speedup vs baseline: 1.0179x; 1.0090x over previous
"""Trainium2 Bass kernel for the bidirectional Mamba-style selective scan.

Problem (B=4, L=2048, D=256, N=16, dt_rank=16):
    x_dbl   = x @ Wx.T                      -> delta_r | Bf | Bb | C
    db_r    = flip(x) @ Wxb.T
    delta   = softplus(delta_r @ Wdt.T + bdt)
    delta_b = softplus(db_r    @ Wdt.T + bdt)
    dA      = exp(delta outer A)            (A = -exp(A_log))
    dBu     = delta*Bf*x + delta_b*Bb*flip(x)
    h_t     = dA_t * h_{t-1} + dBu_t        (scan over L)
    y_t     = einsum(h_t, C_t) ; y += (x + flip(x)) * Dp
    out     = (y @ Wout.T) @ Wadapt.T       -> (B, L)
    returns (out, x)

Sharding: 8 cores = 4 batches x 2 halves of d_inner (128 each).  out_proj is
immediately contracted by Wadapt to one channel, so we fold v = Wadapt @ Wout
(one 256-vector); each core produces y_part[t] = sum_{d in half} v[d]*y[t,d]
and the host adds the two halves per batch (plus the skip term, computed on
host: it is a single dot product with the input).

Per-core device program (d on partitions, t on the free axis, fp32 data with
float32r matmuls -- full-rate on the PE):
  - PE: projections; per-n partition-broadcast of Bf/Bb rows (one-hot selector
    matmul); per-n contraction G_n[t] = sum_d v[d] H_n[d,t] to a (1, 512)
    PSUM row.
  - ACT: softplus (exp + ln -- one shared table set), per-n exp with
    per-partition scale A[:,n]; gathers G rows into partition 32*(n%4) of an
    SBUF tile (engine partition shifts are legal at 32-aligned bases).
  - DVE: dBu products, tensor_tensor_scan recurrence, the small C*G stage.

The sequence flip never happens on device: the host passes both x^T and
flip(x)^T, and every "backward" quantity is computed at forward time from the
flipped copy.
"""

import numpy as np

try:
    import concourse.bass as bass
except ImportError:  # fresh grading dir: repo not on sys.path
    import sys

    sys.path.insert(0, "/opt/trn_rl_repo")
    import concourse.bass as bass

from contextlib import ExitStack

import concourse.bacc as bacc
import concourse.mybir as mybir
import concourse.tile as tile
from concourse.bass_utils import run_bass_kernel_spmd
from concourse.tile_rust import add_dep_helper

L = 2048  # sequence length
DH = 128  # d_inner half handled per core
NST = 16  # state dim n
RK = 16  # dt_rank
CH = 512  # chunk size (one fp32 PSUM bank)
NCH = L // CH

F32 = mybir.dt.float32
F32R = mybir.dt.float32r
AF = mybir.ActivationFunctionType
ALU = mybir.AluOpType


def _f(ap):
    """Reinterpret a float32r AP back as plain fp32 for DVE/ACT reads."""
    return ap.bitcast(mybir.dt.float32)


_BUILT = None


def _build():
    global _BUILT
    if _BUILT is not None:
        return _BUILT

    nc = bacc.Bacc(None, target_bir_lowering=False)

    din = {}
    # wa/wb cols: [0:48 Wx.T | 48:64 Wxb.T | 64:576 wxc] for the two d-halves;
    # w16 cols: [0:2048 one-hot selectors | 2048:2176 Wdt.T];
    # cvec cols: [0 bdt | 1:17 A | 17 v | 18 ones].
    for name, shape in [
        ("xt_a", [DH, L]),  # own d-half of x[b].T
        ("xt_b", [DH, L]),  # other d-half
        ("xr_a", [DH, L]),  # own d-half of flip(x[b]).T
        ("xr_b", [DH, L]),
        ("wa", [DH, 576]),
        ("wb", [DH, 576]),
        ("w16", [NST, NST * DH + DH]),
        ("cvec", [DH, 19]),
    ]:
        din[name] = nc.dram_tensor(name, shape, F32R, kind="ExternalInput")
    y_part = nc.dram_tensor("y_part", [1, L], F32, kind="ExternalOutput")

    with tile.TileContext(nc) as tc, ExitStack() as ctx:
        const = ctx.enter_context(tc.tile_pool(name="const", bufs=1))
        big = ctx.enter_context(tc.tile_pool(name="big", bufs=1))
        work = ctx.enter_context(tc.tile_pool(name="work", bufs=2))

        wts = {}
        for name in ["wa", "wb", "w16", "cvec"]:
            dram = din[name]
            t = const.tile(list(dram.shape), dram.dtype, tag=name, name=f"sb_{name}")
            nc.sync.dma_start(t[:], dram[:])
            wts[name] = t
        wxa, wxb_a, wxca = wts["wa"][:, 0:48], wts["wa"][:, 48:64], wts["wa"][:, 64:576]
        wxb, wxb_b, wxcb = wts["wb"][:, 0:48], wts["wb"][:, 48:64], wts["wb"][:, 64:576]
        sel_all, wdt = wts["w16"][:, 0 : NST * DH], wts["w16"][:, NST * DH :]
        bdt_ap = _f(wts["cvec"][:, 0:1])
        amat = wts["cvec"][:, 1 : 1 + NST]
        vvec = wts["cvec"][:, 17:18]
        ones1 = wts["cvec"][:, 18:19]

        stmp = tc.alloc_tile_pool(name="stmp", bufs=1)
        xs = {}
        for name in ["xt_a", "xt_b", "xr_a", "xr_b"]:
            pool = big if name in ("xt_a", "xr_a") else stmp
            t = pool.tile([DH, L], F32R, tag=name, name=f"sb_{name}")
            nc.sync.dma_start(t[:], din[name][:])
            xs[name] = t

        bfT = big.tile([NST, L], F32R, tag="bfT")
        bbT = big.tile([NST, L], F32R, tag="bbT")
        deltaT = big.tile([DH, L], F32, tag="deltaT")
        pf = big.tile([DH, L], F32, tag="pf")
        pb = big.tile([DH, L], F32, tag="pb")
        # C rows: group g at partitions {0,32,64,96}; built directly by the
        # wxc projection so the unused partitions are exact zeros.
        ctg = [big.tile([DH, L], F32, tag=f"ctg{g}", name=f"ctg{g}") for g in range(4)]

        with tc.tile_pool(name="psetup", bufs=2, space=bass.MemorySpace.PSUM) as psetup:
            spl = stmp.tile([DH, L], F32, tag="spl", name="spl", bufs=1)
            for c in range(NCH):
                cs = slice(c * CH, (c + 1) * CH)
                for gi, dst in enumerate([None, bfT, bbT]):
                    gs = slice(gi * 16, (gi + 1) * 16)
                    p_xdbl = psetup.tile([16, CH], F32, tag="p_xdbl")
                    nc.tensor.matmul(p_xdbl[:], wxa[:, gs], xs["xt_a"][:, cs], start=True, stop=False)
                    nc.tensor.matmul(p_xdbl[:], wxb[:, gs], xs["xt_b"][:, cs], start=False, stop=True)
                    if dst is None:
                        drT_c = stmp.tile([RK, CH], F32R, tag="drT", name="drT_c", bufs=2)
                        nc.scalar.copy(drT_c[:], p_xdbl[:])
                        p_dd = psetup.tile([DH, CH], F32, tag="p_dd")
                        nc.tensor.matmul(p_dd[:], wdt[:], drT_c[:], start=True, stop=True)
                        last_fexp = nc.scalar.activation(spl[:, cs], p_dd[:], AF.Exp, bias=bdt_ap, scale=1.0)
                    else:
                        nc.scalar.copy(dst[:, cs], p_xdbl[:])
            for c in range(NCH):
                cs = slice(c * CH, (c + 1) * CH)
                ln_i = nc.scalar.activation(deltaT[:, cs], spl[:, cs], AF.Ln, bias=1.0, scale=1.0)
                if c == 0:
                    add_dep_helper(ln_i.ins, last_fexp.ins, reason="act-table batching fwd")
            for g in range(4):
                ggs = slice(g * DH, (g + 1) * DH)
                for c in range(NCH):
                    cs = slice(c * CH, (c + 1) * CH)
                    p_ct = psetup.tile([DH, CH], F32, tag="p_ct")
                    nc.tensor.matmul(p_ct[:], wxca[:, ggs], xs["xt_a"][:, cs], start=True, stop=False)
                    nc.tensor.matmul(p_ct[:], wxcb[:, ggs], xs["xt_b"][:, cs], start=False, stop=True)
                    nc.scalar.copy(ctg[g][:, cs], p_ct[:])
            # softplus(z) = ln(exp(z) + 1); exp and ln live in one ACT table
            # set (no softplus table exists in this compiler).
            for c in range(NCH):
                cs = slice(c * CH, (c + 1) * CH)
                p_dbr = psetup.tile([RK, CH], F32, tag="p_dbr")
                nc.tensor.matmul(p_dbr[:], wxb_a[:], xs["xr_a"][:, cs], start=True, stop=False)
                nc.tensor.matmul(p_dbr[:], wxb_b[:], xs["xr_b"][:, cs], start=False, stop=True)
                dbr_c = stmp.tile([RK, CH], F32R, tag="dbr", name="dbr_c", bufs=2)
                nc.scalar.copy(dbr_c[:], p_dbr[:])
                p_dd = psetup.tile([DH, CH], F32, tag="p_dd")
                nc.tensor.matmul(p_dd[:], wdt[:], dbr_c[:], start=True, stop=True)
                last_bexp = nc.scalar.activation(spl[:, cs], p_dd[:], AF.Exp, bias=bdt_ap, scale=1.0)
            last_ln = None
            for c in range(NCH):
                cs = slice(c * CH, (c + 1) * CH)
                dlb_c = stmp.tile([DH, CH], F32, tag="dlb", name="dlb_c", bufs=2)
                last_ln = nc.scalar.activation(dlb_c[:], spl[:, cs], AF.Ln, bias=1.0, scale=1.0)
                if c == 0:
                    add_dep_helper(last_ln.ins, last_bexp.ins, reason="act-table batching bwd")
                nc.vector.tensor_mul(pb[:, cs], dlb_c[:], _f(xs["xr_a"][:, cs]))

        nc.vector.tensor_mul(pf[:], deltaT[:], _f(xs["xt_a"][:]))

        ysb = big.tile([1, L], F32, tag="ysb")
        nc.vector.memset(ysb[:], 0.0)
        stmp.release()

        with (
            tc.tile_pool(name="pbc", bufs=2, space=bass.MemorySpace.PSUM) as pbc,
            tc.tile_pool(name="pgp", bufs=2, space=bass.MemorySpace.PSUM) as pgp,
            tc.tile_pool(name="pyg", bufs=2, space=bass.MemorySpace.PSUM) as pyg,
        ):
            # G rows gather tile: group g writes rows {0,32,64,96}; all other
            # rows are zeroed once and multiply against exact-zero ctg rows.
            gsb = work.tile([DH, L], F32, tag="gsb", name="gsb", bufs=1)
            nc.vector.memset(gsb[:], 0.0)
            cg = work.tile([DH, L], F32R, tag="cg", name="cg", bufs=1)
            for g in range(4):
                for k in range(4):
                    n = 4 * g + k
                    seln = sel_all[:, n * DH : (n + 1) * DH]
                    h_prev = None
                    for j in range(2):
                        FD = 2 * CH
                        js = slice(j * FD, (j + 1) * FD)
                        dA = work.tile([DH, FD], F32, tag="dA", name="dA", bufs=3)
                        exp_i = nc.scalar.activation(
                            dA[:], deltaT[:, js], AF.Exp, bias=0.0, scale=_f(amat[:, n : n + 1])
                        )
                        if n == 0 and j == 0 and last_ln is not None:
                            add_dep_helper(exp_i.ins, last_ln.ins, reason="act-table batching")
                        bcf = pbc.tile([DH, FD], F32, tag="bc", name="bcf")
                        for h in range(2):
                            hs = slice(h * CH, (h + 1) * CH)
                            nc.tensor.matmul(bcf[:, hs], seln, bfT[:, j * FD + h * CH : j * FD + (h + 1) * CH])
                        w1 = work.tile([DH, FD], F32, tag="w1", name="w1")
                        nc.vector.tensor_mul(w1[:], pf[:, js], bcf[:])
                        bcb = pbc.tile([DH, FD], F32, tag="bc", name="bcb")
                        for h in range(2):
                            hs = slice(h * CH, (h + 1) * CH)
                            nc.tensor.matmul(bcb[:, hs], seln, bbT[:, j * FD + h * CH : j * FD + (h + 1) * CH])
                        w2 = work.tile([DH, FD], F32, tag="w2", name="w2")
                        nc.vector.tensor_mul(w2[:], pb[:, js], bcb[:])
                        dbu = work.tile([DH, FD], F32, tag="dbu", name="dbu")
                        nc.vector.tensor_add(dbu[:], w1[:], w2[:])
                        hs_t = work.tile([DH, FD], F32R, tag="h", name="hst")
                        init = 0.0 if j == 0 else _f(h_prev[:, FD - 1 : FD])
                        nc.vector.tensor_tensor_scan(
                            hs_t[:], dA[:], dbu[:], init, ALU.mult, ALU.add
                        )
                        h_prev = hs_t
                        for h in range(2):
                            c = 2 * j + h
                            cs = slice(c * CH, (c + 1) * CH)
                            hhs = slice(h * CH, (h + 1) * CH)
                            gp1 = pgp.tile([1, CH], F32, tag="gp1", name="gp1", bufs=3)
                            nc.tensor.matmul(gp1[:], vvec, hs_t[:, hhs])
                            nc.scalar.copy(gsb[32 * k : 32 * k + 1, cs], gp1[:])
                # multiply the gathered G rows by C, reduce over d and n
                nc.vector.tensor_mul(cg[:], ctg[g][:], gsb[:])
                for c in range(NCH):
                    cs = slice(c * CH, (c + 1) * CH)
                    yg = pyg.tile([1, CH], F32, tag="yg", name=f"yg{g}_{c}", bufs=1)
                    nc.tensor.matmul(yg[:], ones1, cg[:, cs])
                    nc.vector.tensor_add(ysb[:, cs], ysb[:, cs], yg[:])

            nc.sync.dma_start(y_part[:], ysb[:])

    nc.compile()
    _BUILT = nc
    return nc


def _shard_inputs(x, Wx, Wxb, Wdt, bdt, A_log, Dp, Wout, Wadapt):
    x = np.asarray(x, dtype=np.float32)
    Wx = np.asarray(Wx, dtype=np.float32)
    Wxb = np.asarray(Wxb, dtype=np.float32)
    Wdt = np.asarray(Wdt, dtype=np.float32)
    bdt = np.asarray(bdt, dtype=np.float32)
    A = -np.exp(np.asarray(A_log, dtype=np.float32))  # (D, N)
    Wout = np.asarray(Wout, dtype=np.float32)
    Wadapt = np.asarray(Wadapt, dtype=np.float32)

    v = (Wadapt @ Wout)[0]  # (D,)
    WxT = np.ascontiguousarray(Wx.T)  # (D, 64)
    WxbT = np.ascontiguousarray(Wxb.T)  # (D, R)
    WdtT = np.ascontiguousarray(Wdt.T)  # (R, D)
    # wxc[d, g*128 + 32*k] = Wx[48 + 4g + k, d]
    wxc = np.zeros((2 * DH, 4 * DH), dtype=np.float32)
    for g in range(4):
        for k in range(4):
            wxc[:, g * DH + 32 * k] = Wx[48 + 4 * g + k, :]

    in_maps = []
    for cidx in range(8):
        bi, hi = cidx // 2, cidx % 2
        sa = slice(hi * DH, hi * DH + DH)
        sb = slice((1 - hi) * DH, (1 - hi) * DH + DH)
        xT = np.ascontiguousarray(x[bi].T)  # (D, L)
        xrT = np.ascontiguousarray(x[bi, ::-1].T)
        wa = np.concatenate([WxT[sa, :48], WxbT[sa], wxc[sa]], axis=1)
        wb = np.concatenate([WxT[sb, :48], WxbT[sb], wxc[sb]], axis=1)
        w16 = np.zeros((NST, NST * DH + DH), dtype=np.float32)
        for n in range(NST):
            w16[n, n * DH : (n + 1) * DH] = 1.0
        w16[:, NST * DH :] = WdtT[:, sa]
        cvec = np.zeros((DH, 19), dtype=np.float32)
        cvec[:, 0] = bdt[sa]
        cvec[:, 1 : 1 + NST] = A[sa]
        cvec[:, 17] = v[sa]
        cvec[:, 18] = 1.0
        in_maps.append(
            {
                "xt_a": np.ascontiguousarray(xT[sa]),
                "xt_b": np.ascontiguousarray(xT[sb]),
                "xr_a": np.ascontiguousarray(xrT[sa]),
                "xr_b": np.ascontiguousarray(xrT[sb]),
                "wa": np.ascontiguousarray(wa),
                "wb": np.ascontiguousarray(wb),
                "w16": w16,
                "cvec": cvec,
            }
        )
    return in_maps


def kernel(x, Wx, Wxb, Wdt, bdt, A_log, Dp, Wout, Wadapt, _run_kwargs=None):
    nc = _build()
    x = np.asarray(x, dtype=np.float32)
    in_maps = _shard_inputs(x, Wx, Wxb, Wdt, bdt, A_log, Dp, Wout, Wadapt)
    kwargs = dict(_run_kwargs or {})
    res = run_bass_kernel_spmd(nc, in_maps, core_ids=list(range(8)), **kwargs)
    parts = [r["y_part"].reshape(L) for r in res.results]
    # skip path (y += (x + flip(x)) * Dp before out_proj) reduces to one dot
    # with vd = (Wadapt @ Wout) * Dp; computed on host.
    v = (np.asarray(Wadapt, np.float32) @ np.asarray(Wout, np.float32))[0]
    vd = v * np.asarray(Dp, np.float32)
    y = np.empty((4, L), dtype=np.float32)
    for b in range(4):
        t = x[b] @ vd
        y[b] = parts[2 * b] + parts[2 * b + 1] + t + t[::-1]
    out = (y, x)
    if _run_kwargs is not None:
        return out, res
    return out


# revision 31
# speedup vs baseline: 1.0205x; 1.0025x over previous
"""Trainium2 Bass kernel for the bidirectional Mamba-style selective scan.

Problem (B=4, L=2048, D=256, N=16, dt_rank=16):
    x_dbl   = x @ Wx.T                      -> delta_r | Bf | Bb | C
    db_r    = flip(x) @ Wxb.T
    delta   = softplus(delta_r @ Wdt.T + bdt)
    delta_b = softplus(db_r    @ Wdt.T + bdt)
    dA      = exp(delta outer A)            (A = -exp(A_log))
    dBu     = delta*Bf*x + delta_b*Bb*flip(x)
    h_t     = dA_t * h_{t-1} + dBu_t        (scan over L)
    y_t     = einsum(h_t, C_t) ; y += (x + flip(x)) * Dp
    out     = (y @ Wout.T) @ Wadapt.T       -> (B, L)
    returns (out, x)

Sharding: 8 cores = 4 batches x 2 halves of d_inner (128 each).  out_proj is
immediately contracted by Wadapt to one channel, so we fold v = Wadapt @ Wout
(one 256-vector); each core produces y_part[t] = sum_{d in half} v[d]*y[t,d]
and the host adds the two halves per batch (plus the skip term, computed on
host: it is a single dot product with the input).

Per-core device program (d on partitions, t on the free axis, fp32 data with
float32r matmuls -- full-rate on the PE):
  - PE: projections; per-n partition-broadcast of Bf/Bb rows (one-hot selector
    matmul); per-n contraction G_n[t] = sum_d v[d] H_n[d,t] to a (1, 512)
    PSUM row.
  - ACT: softplus (exp + ln -- one shared table set), per-n exp with
    per-partition scale A[:,n]; gathers G rows into partition 32*(n%4) of an
    SBUF tile (engine partition shifts are legal at 32-aligned bases).
  - DVE: dBu products, tensor_tensor_scan recurrence, the small C*G stage.

The sequence flip never happens on device: the host passes both x^T and
flip(x)^T, and every "backward" quantity is computed at forward time from the
flipped copy.
"""

import numpy as np

try:
    import concourse.bass as bass
except ImportError:  # fresh grading dir: repo not on sys.path
    import sys

    sys.path.insert(0, "/opt/trn_rl_repo")
    import concourse.bass as bass

from contextlib import ExitStack

import concourse.bacc as bacc
import concourse.mybir as mybir
import concourse.tile as tile
from concourse.bass_utils import run_bass_kernel_spmd
from concourse.tile_rust import add_dep_helper

L = 2048  # sequence length
DH = 128  # d_inner half handled per core
NST = 16  # state dim n
RK = 16  # dt_rank
CH = 512  # chunk size (one fp32 PSUM bank)
NCH = L // CH

F32 = mybir.dt.float32
F32R = mybir.dt.float32r
AF = mybir.ActivationFunctionType
ALU = mybir.AluOpType


def _f(ap):
    """Reinterpret a float32r AP back as plain fp32 for DVE/ACT reads."""
    return ap.bitcast(mybir.dt.float32)


_BUILT = None


def _build():
    global _BUILT
    if _BUILT is not None:
        return _BUILT

    nc = bacc.Bacc(None, target_bir_lowering=False)

    din = {}
    # wa/wb cols: [0:48 Wx.T | 48:64 Wxb.T | 64:576 wxc] for the two d-halves;
    # w16 cols: [0:2048 one-hot selectors | 2048:2176 Wdt.T];
    # cvec cols: [0 bdt | 1:17 A | 17 v | 18 ones].
    for name, shape in [
        ("xt_a", [DH, L]),  # own d-half of x[b].T
        ("xt_b", [DH, L]),  # other d-half
        ("xr_a", [DH, L]),  # own d-half of flip(x[b]).T
        ("xr_b", [DH, L]),
        ("wa", [DH, 576]),
        ("wb", [DH, 576]),
        ("w16", [NST, NST * DH + DH]),
        ("cvec", [DH, 19]),
    ]:
        din[name] = nc.dram_tensor(name, shape, F32R, kind="ExternalInput")
    y_part = nc.dram_tensor("y_part", [1, L], F32, kind="ExternalOutput")

    with tile.TileContext(nc) as tc, ExitStack() as ctx:
        const = ctx.enter_context(tc.tile_pool(name="const", bufs=1))
        big = ctx.enter_context(tc.tile_pool(name="big", bufs=1))
        work = ctx.enter_context(tc.tile_pool(name="work", bufs=2))

        wts = {}
        for name in ["wa", "wb", "w16", "cvec"]:
            dram = din[name]
            t = const.tile(list(dram.shape), dram.dtype, tag=name, name=f"sb_{name}")
            nc.sync.dma_start(t[:], dram[:])
            wts[name] = t
        wxa, wxb_a, wxca = wts["wa"][:, 0:48], wts["wa"][:, 48:64], wts["wa"][:, 64:576]
        wxb, wxb_b, wxcb = wts["wb"][:, 0:48], wts["wb"][:, 48:64], wts["wb"][:, 64:576]
        sel_all, wdt = wts["w16"][:, 0 : NST * DH], wts["w16"][:, NST * DH :]
        bdt_ap = _f(wts["cvec"][:, 0:1])
        amat = wts["cvec"][:, 1 : 1 + NST]
        vvec = wts["cvec"][:, 17:18]
        ones1 = wts["cvec"][:, 18:19]

        stmp = tc.alloc_tile_pool(name="stmp", bufs=1)
        xs = {}
        for name in ["xt_a", "xt_b", "xr_a", "xr_b"]:
            pool = big if name in ("xt_a", "xr_a") else stmp
            t = pool.tile([DH, L], F32R, tag=name, name=f"sb_{name}")
            nc.sync.dma_start(t[:], din[name][:])
            xs[name] = t

        bfT = big.tile([NST, L], F32R, tag="bfT")
        bbT = big.tile([NST, L], F32R, tag="bbT")
        deltaT = big.tile([DH, L], F32, tag="deltaT")
        pf = big.tile([DH, L], F32, tag="pf")
        pb = big.tile([DH, L], F32, tag="pb")
        # C rows: group g at partitions {0,32,64,96}; built directly by the
        # wxc projection so the unused partitions are exact zeros.
        ctg = [big.tile([DH, L], F32, tag=f"ctg{g}", name=f"ctg{g}") for g in range(4)]

        with tc.tile_pool(name="psetup", bufs=2, space=bass.MemorySpace.PSUM) as psetup:
            spl = stmp.tile([DH, L], F32, tag="spl", name="spl", bufs=1)
            for c in range(NCH):
                cs = slice(c * CH, (c + 1) * CH)
                for gi, dst in enumerate([None, bfT, bbT]):
                    gs = slice(gi * 16, (gi + 1) * 16)
                    p_xdbl = psetup.tile([16, CH], F32, tag="p_xdbl")
                    nc.tensor.matmul(p_xdbl[:], wxa[:, gs], xs["xt_a"][:, cs], start=True, stop=False)
                    nc.tensor.matmul(p_xdbl[:], wxb[:, gs], xs["xt_b"][:, cs], start=False, stop=True)
                    if dst is None:
                        drT_c = stmp.tile([RK, CH], F32R, tag="drT", name="drT_c", bufs=2)
                        nc.scalar.copy(drT_c[:], p_xdbl[:])
                        p_dd = psetup.tile([DH, CH], F32, tag="p_dd")
                        nc.tensor.matmul(p_dd[:], wdt[:], drT_c[:], start=True, stop=True)
                        last_fexp = nc.scalar.activation(spl[:, cs], p_dd[:], AF.Exp, bias=bdt_ap, scale=1.0)
                    else:
                        nc.scalar.copy(dst[:, cs], p_xdbl[:])
            for c in range(NCH):
                cs = slice(c * CH, (c + 1) * CH)
                ln_i = nc.scalar.activation(deltaT[:, cs], spl[:, cs], AF.Ln, bias=1.0, scale=1.0)
                if c == 0:
                    add_dep_helper(ln_i.ins, last_fexp.ins, reason="act-table batching fwd")
            for g in range(4):
                ggs = slice(g * DH, (g + 1) * DH)
                for c in range(NCH):
                    cs = slice(c * CH, (c + 1) * CH)
                    p_ct = psetup.tile([DH, CH], F32, tag="p_ct")
                    nc.tensor.matmul(p_ct[:], wxca[:, ggs], xs["xt_a"][:, cs], start=True, stop=False)
                    nc.tensor.matmul(p_ct[:], wxcb[:, ggs], xs["xt_b"][:, cs], start=False, stop=True)
                    nc.scalar.copy(ctg[g][:, cs], p_ct[:])
            # softplus(z) = ln(exp(z) + 1); exp and ln live in one ACT table
            # set (no softplus table exists in this compiler).
            for c in range(NCH):
                cs = slice(c * CH, (c + 1) * CH)
                p_dbr = psetup.tile([RK, CH], F32, tag="p_dbr")
                nc.tensor.matmul(p_dbr[:], wxb_a[:], xs["xr_a"][:, cs], start=True, stop=False)
                nc.tensor.matmul(p_dbr[:], wxb_b[:], xs["xr_b"][:, cs], start=False, stop=True)
                dbr_c = stmp.tile([RK, CH], F32R, tag="dbr", name="dbr_c", bufs=2)
                nc.scalar.copy(dbr_c[:], p_dbr[:])
                p_dd = psetup.tile([DH, CH], F32, tag="p_dd")
                nc.tensor.matmul(p_dd[:], wdt[:], dbr_c[:], start=True, stop=True)
                last_bexp = nc.scalar.activation(spl[:, cs], p_dd[:], AF.Exp, bias=bdt_ap, scale=1.0)
            last_ln = None
            for c in range(NCH):
                cs = slice(c * CH, (c + 1) * CH)
                dlb_c = stmp.tile([DH, CH], F32, tag="dlb", name="dlb_c", bufs=2)
                last_ln = nc.scalar.activation(dlb_c[:], spl[:, cs], AF.Ln, bias=1.0, scale=1.0)
                if c == 0:
                    add_dep_helper(last_ln.ins, last_bexp.ins, reason="act-table batching bwd")
                nc.vector.tensor_mul(pb[:, cs], dlb_c[:], _f(xs["xr_a"][:, cs]))

        nc.vector.tensor_mul(pf[:], deltaT[:], _f(xs["xt_a"][:]))

        ysb = big.tile([1, L], F32, tag="ysb")
        nc.vector.memset(ysb[:], 0.0)
        stmp.release()

        with (
            tc.tile_pool(name="pbc", bufs=2, space=bass.MemorySpace.PSUM) as pbc,
            tc.tile_pool(name="pgp", bufs=2, space=bass.MemorySpace.PSUM) as pgp,
            tc.tile_pool(name="pyg", bufs=2, space=bass.MemorySpace.PSUM) as pyg,
        ):
            # G rows gather tile: group g writes rows {0,32,64,96}; all other
            # rows are zeroed once and multiply against exact-zero ctg rows.
            gsb = work.tile([DH, L], F32, tag="gsb", name="gsb", bufs=1)
            nc.vector.memset(gsb[:], 0.0)
            cg = work.tile([DH, L], F32R, tag="cg", name="cg", bufs=1)
            for g in range(4):
                for k in range(4):
                    n = 4 * g + k
                    seln = sel_all[:, n * DH : (n + 1) * DH]
                    h_prev = None
                    for j in range(2):
                        FD = 2 * CH
                        js = slice(j * FD, (j + 1) * FD)
                        dA = work.tile([DH, FD], F32, tag="dA", name="dA", bufs=3)
                        exp_i = nc.scalar.activation(
                            dA[:], deltaT[:, js], AF.Exp, bias=0.0, scale=_f(amat[:, n : n + 1])
                        )
                        if n == 0 and j == 0 and last_ln is not None:
                            add_dep_helper(exp_i.ins, last_ln.ins, reason="act-table batching")
                        bcf = pbc.tile([DH, FD], F32, tag="bc", name="bcf")
                        for h in range(2):
                            hs = slice(h * CH, (h + 1) * CH)
                            nc.tensor.matmul(bcf[:, hs], seln, bfT[:, j * FD + h * CH : j * FD + (h + 1) * CH])
                        w1 = work.tile([DH, FD], F32, tag="w1", name="w1")
                        nc.vector.tensor_mul(w1[:], pf[:, js], bcf[:])
                        bcb = pbc.tile([DH, FD], F32, tag="bc", name="bcb")
                        for h in range(2):
                            hs = slice(h * CH, (h + 1) * CH)
                            nc.tensor.matmul(bcb[:, hs], seln, bbT[:, j * FD + h * CH : j * FD + (h + 1) * CH])
                        w2 = work.tile([DH, FD], F32, tag="w2", name="w2")
                        nc.vector.tensor_mul(w2[:], pb[:, js], bcb[:])
                        dbu = work.tile([DH, FD], F32, tag="dbu", name="dbu")
                        nc.vector.tensor_add(dbu[:], w1[:], w2[:])
                        hs_t = work.tile([DH, FD], F32R, tag="h", name="hst")
                        init = 0.0 if j == 0 else _f(h_prev[:, FD - 1 : FD])
                        nc.vector.tensor_tensor_scan(
                            hs_t[:], dA[:], dbu[:], init, ALU.mult, ALU.add
                        )
                        h_prev = hs_t
                        for h in range(2):
                            c = 2 * j + h
                            cs = slice(c * CH, (c + 1) * CH)
                            hhs = slice(h * CH, (h + 1) * CH)
                            gp1 = pgp.tile([1, CH], F32, tag="gp1", name="gp1", bufs=3)
                            nc.tensor.matmul(gp1[:], vvec, hs_t[:, hhs])
                            nc.scalar.copy(gsb[32 * k : 32 * k + 1, cs], gp1[:])
                # multiply the gathered G rows by C, reduce over d and n;
                # chunked so each reduction starts as soon as its piece is ready
                for c in range(NCH):
                    cs = slice(c * CH, (c + 1) * CH)
                    nc.vector.tensor_mul(cg[:, cs], ctg[g][:, cs], gsb[:, cs])
                    yg = pyg.tile([1, CH], F32, tag="yg", name=f"yg{g}_{c}", bufs=1)
                    nc.tensor.matmul(yg[:], ones1, cg[:, cs])
                    nc.vector.tensor_add(ysb[:, cs], ysb[:, cs], yg[:])

            nc.sync.dma_start(y_part[:], ysb[:])

    nc.compile()
    _BUILT = nc
    return nc


def _shard_inputs(x, Wx, Wxb, Wdt, bdt, A_log, Dp, Wout, Wadapt):
    x = np.asarray(x, dtype=np.float32)
    Wx = np.asarray(Wx, dtype=np.float32)
    Wxb = np.asarray(Wxb, dtype=np.float32)
    Wdt = np.asarray(Wdt, dtype=np.float32)
    bdt = np.asarray(bdt, dtype=np.float32)
    A = -np.exp(np.asarray(A_log, dtype=np.float32))  # (D, N)
    Wout = np.asarray(Wout, dtype=np.float32)
    Wadapt = np.asarray(Wadapt, dtype=np.float32)

    v = (Wadapt @ Wout)[0]  # (D,)
    WxT = np.ascontiguousarray(Wx.T)  # (D, 64)
    WxbT = np.ascontiguousarray(Wxb.T)  # (D, R)
    WdtT = np.ascontiguousarray(Wdt.T)  # (R, D)
    # wxc[d, g*128 + 32*k] = Wx[48 + 4g + k, d]
    wxc = np.zeros((2 * DH, 4 * DH), dtype=np.float32)
    for g in range(4):
        for k in range(4):
            wxc[:, g * DH + 32 * k] = Wx[48 + 4 * g + k, :]

    in_maps = []
    for cidx in range(8):
        bi, hi = cidx // 2, cidx % 2
        sa = slice(hi * DH, hi * DH + DH)
        sb = slice((1 - hi) * DH, (1 - hi) * DH + DH)
        xT = np.ascontiguousarray(x[bi].T)  # (D, L)
        xrT = np.ascontiguousarray(x[bi, ::-1].T)
        wa = np.concatenate([WxT[sa, :48], WxbT[sa], wxc[sa]], axis=1)
        wb = np.concatenate([WxT[sb, :48], WxbT[sb], wxc[sb]], axis=1)
        w16 = np.zeros((NST, NST * DH + DH), dtype=np.float32)
        for n in range(NST):
            w16[n, n * DH : (n + 1) * DH] = 1.0
        w16[:, NST * DH :] = WdtT[:, sa]
        cvec = np.zeros((DH, 19), dtype=np.float32)
        cvec[:, 0] = bdt[sa]
        cvec[:, 1 : 1 + NST] = A[sa]
        cvec[:, 17] = v[sa]
        cvec[:, 18] = 1.0
        in_maps.append(
            {
                "xt_a": np.ascontiguousarray(xT[sa]),
                "xt_b": np.ascontiguousarray(xT[sb]),
                "xr_a": np.ascontiguousarray(xrT[sa]),
                "xr_b": np.ascontiguousarray(xrT[sb]),
                "wa": np.ascontiguousarray(wa),
                "wb": np.ascontiguousarray(wb),
                "w16": w16,
                "cvec": cvec,
            }
        )
    return in_maps


def kernel(x, Wx, Wxb, Wdt, bdt, A_log, Dp, Wout, Wadapt, _run_kwargs=None):
    nc = _build()
    x = np.asarray(x, dtype=np.float32)
    in_maps = _shard_inputs(x, Wx, Wxb, Wdt, bdt, A_log, Dp, Wout, Wadapt)
    kwargs = dict(_run_kwargs or {})
    res = run_bass_kernel_spmd(nc, in_maps, core_ids=list(range(8)), **kwargs)
    parts = [r["y_part"].reshape(L) for r in res.results]
    # skip path (y += (x + flip(x)) * Dp before out_proj) reduces to one dot
    # with vd = (Wadapt @ Wout) * Dp; computed on host.
    v = (np.asarray(Wadapt, np.float32) @ np.asarray(Wout, np.float32))[0]
    vd = v * np.asarray(Dp, np.float32)
    y = np.empty((4, L), dtype=np.float32)
    for b in range(4):
        t = x[b] @ vd
        y[b] = parts[2 * b] + parts[2 * b + 1] + t + t[::-1]
    out = (y, x)
    if _run_kwargs is not None:
        return out, res
    return out
